# revision 3
# baseline (speedup 1.0000x reference)
"""APPNP layer (GNN message passing) on 8 TRN2 NeuronCores — Design B.

support = x @ W; h = support; 10x: h = relu(0.9*SpMM(A,h) + 0.1*support)

Distribution: dst-shard nodes across 8 cores (6250 each), with a per-core
node relabeling that balances each of the 49 blocks of 128 nodes on
(lo, hi) in-degree.  Per iteration:
  - AllGather the 8 bf16 shard tables into a DRAM replica (rows padded to
    128 cols; row of local node n = (n%128)*NB + n//128, i.e. node n lives
    at partition n%128, block n//128);
  - edge gather: per (block, half) the in-edges are packed into 128-slot
    tiles; all lo tiles (block-major), then all hi tiles; gathers run in
    chunks of 32 tiles via gpsimd.dma_gather from the DRAM replica halves;
  - segment-sum: per tile one matmul, lhsT = host-built one-hot
    [128 slots, 128 node-cols] bf16 carrying 0.9*val, rhs = gathered rows
    [128 slots, :96]; accumulated in PSUM across the block's tiles
    (start/stop flags); one-hots streamed from DRAM per chunk;
  - lo epilogue per block: acc = psum + 0.1*support (f32, SBUF);
    hi epilogue: d = psum + acc; relu -> next h (bf16) or final out (f32).

No staging round-trips, no boundary gathers, no prefix-sum trick.
kernel(**inputs) accepts FULL inputs and returns the FULL [50000, 96] output.
"""

import numpy as np

_DEF = dict(N=50000, E=800000, IN_F=512, OUT_F=96, ALPHA=0.1, ITERS=10, CORES=8)

OUT_F = 96
FW = 128          # padded feature width of the h table (256B bf16 rows)
P = 128
CH_TILES = 32     # tiles per gather chunk (4096 indices/call)


def _wrap16(idx):
    """[n] int -> dma_gather idx layout [128, n//16] int16."""
    n = idx.shape[0]
    assert n % 16 == 0
    return np.tile(idx.reshape(n // 16, 16).T, (8, 1)).astype(np.int16)


# ----------------------------------------------------------------------------
# Host-side preprocessing
# ----------------------------------------------------------------------------

def _balance_blocks(d_lo, d_hi, SHARD, NB):
    """Assign SHARD nodes to NB blocks of <=128, balancing lo/hi degree.
    Returns perm[SP]: perm[p*NB+b] = old local node at (partition p, block b),
    -1 for padding."""
    SP = NB * P
    order = np.argsort(-(d_lo + d_hi), kind="stable")
    blk_lo = np.zeros(NB, np.int64)
    blk_hi = np.zeros(NB, np.int64)
    blk_cnt = np.zeros(NB, np.int64)
    members = [[] for _ in range(NB)]
    for node in order:
        cand = np.nonzero(blk_cnt < P)[0]
        load = np.maximum(blk_lo[cand] + d_lo[node], blk_hi[cand] + d_hi[node])
        j = cand[np.argmin(load)]
        members[j].append(node)
        blk_lo[j] += d_lo[node]
        blk_hi[j] += d_hi[node]
        blk_cnt[j] += 1
    perm = np.full(SP, -1, np.int64)
    for b in range(NB):
        for i, node in enumerate(members[b]):
            perm[i * NB + b] = node
    return perm


def _prep(inputs, cfg):
    import ml_dtypes
    bf16 = ml_dtypes.bfloat16

    N, IN_F, C = cfg["N"], cfg["IN_F"], cfg["CORES"]
    ALPHA = cfg["ALPHA"]
    x = np.asarray(inputs["x"], np.float32)
    w = np.asarray(inputs["weight"], np.float32)
    src = np.asarray(inputs["edge_src"], np.int64)
    dst = np.asarray(inputs["edge_dst"], np.int64)
    val = np.asarray(inputs["edge_val"], np.float32)

    SHARD = N // C
    NB = -(-SHARD // P)
    SP = NB * P
    HALF = (C // 2) * SP

    owner_dst = dst // SHARD
    local_dst = dst % SHARD
    is_hi_src = (src // SHARD) >= C // 2

    # --- per-core block balancing; build global table-row mapping
    perms = []
    rows_of = np.zeros(N, np.int64)       # old global node -> global table row
    for c in range(C):
        m = owner_dst == c
        ld = local_dst[m]
        d_lo = np.bincount(ld[~is_hi_src[m]], minlength=SHARD)
        d_hi = np.bincount(ld[is_hi_src[m]], minlength=SHARD)
        perm = _balance_blocks(d_lo, d_hi, SHARD, NB)
        perms.append(perm)
        valid = perm >= 0
        rows_of[c * SHARD + perm[valid]] = c * SP + np.nonzero(valid)[0]
    grow = rows_of[src]                   # per-edge table row of src
    e_hi = grow >= HALF

    # --- per (core, block, half) edge lists + global tile schedule
    edges_by = {}
    tc_lo = np.zeros((C, NB), np.int64)
    tc_hi = np.zeros((C, NB), np.int64)
    for c in range(C):
        m = owner_dst == c
        perm = perms[c]
        pos = np.zeros(SHARD, np.int64)
        valid = perm >= 0
        pos[perm[valid]] = np.nonzero(valid)[0]
        rp = pos[local_dst[m]]            # table-row position within core
        pb = rp // NB                     # partition = one-hot column
        bb = rp % NB                      # block
        gs = grow[m]
        hi = e_hi[m]
        vv = val[m]
        order = np.argsort(bb * 2 + hi, kind="stable")
        gs, pb, bb, hi, vv = gs[order], pb[order], bb[order], hi[order], vv[order]
        # split indices per (block, half)
        for b in range(NB):
            selb = bb == b
            for h in (0, 1):
                sel = selb & (hi == bool(h))
                edges_by[(c, b, h)] = (gs[sel] - h * HALF, pb[sel], vv[sel])
                cnt = -(-int(sel.sum()) // P)
                (tc_lo if h == 0 else tc_hi)[c, b] = cnt
    TL = tc_lo.max(axis=0)                # global per-block lo tile counts
    TH = tc_hi.max(axis=0)
    assert TL.min() > 0 and TH.min() > 0, "empty block-half; epilogue invariant"
    T = int(TL.sum() + TH.sum())
    TLs = int(TL.sum())

    # chunking: lo stream [0, TLs), hi stream [TLs, T); chunks within stream.
    # Taper the hi-stream tail (16/8/4) so the final drain is short and
    # overlaps the preceding chunk's generation.
    chunks = []                           # (tile_start, ntiles, half)
    for h, lo, hi_ in ((0, 0, TLs), (1, TLs, T)):
        t = lo
        rem = hi_ - lo
        taper = [16, 8, 4] if h == 1 else []
        body = rem - sum(taper) if rem > sum(taper) + CH_TILES else rem
        while t < lo + body:
            n = min(CH_TILES, lo + body - t)
            chunks.append((t, n, h))
            t += n
        for n in taper:
            if t < hi_:
                n = min(n, hi_ - t)
                chunks.append((t, n, h))
                t += n

    # per-tile matmul schedule (same for all cores)
    sched = []                            # (block, start, stop)
    for TBL in (TL, TH):
        for b in range(NB):
            n = int(TBL[b])
            for t in range(n):
                sched.append((b, t == 0, t == n - 1))
    assert len(sched) == T

    # --- per-core tile data: idx + one-hot
    in_maps = []
    wp = np.zeros((P, (IN_F // P) * OUT_F), np.float32)
    for k in range(IN_F // P):
        wp[:, k * OUT_F:(k + 1) * OUT_F] = w[k * P:(k + 1) * P, :]

    iot = np.tile(np.arange(P, dtype=np.float32), (P, 1))
    for c in range(C):
        idx = np.zeros((T, P), np.int64)
        w1c = np.zeros((T, P), np.float32)     # one-hot column per slot
        w1v = np.zeros((T, P), np.float32)     # 0.9*val per slot
        tpos = 0
        for h, TBL in ((0, TL), (1, TH)):
            for b in range(NB):
                rows, cols, vals = edges_by[(c, b, h)]
                nt = int(TBL[b])
                for t in range(nt):
                    sl = slice(t * P, min((t + 1) * P, len(rows)))
                    r = rows[sl]
                    k = len(r)
                    if k:
                        idx[tpos + t, :k] = r
                        w1c[tpos + t, :k] = cols[sl]
                        w1v[tpos + t, :k] = (1.0 - ALPHA) * vals[sl]
                tpos += nt
        # gather idx arrays per chunk
        idxg = np.concatenate(
            [_wrap16(idx[t0:t0 + n].ravel()) for t0, n, h in chunks], axis=1)

        lo, hi_ = c * SHARD, (c + 1) * SHARD
        perm = perms[c]
        xt = np.zeros((IN_F, SP), np.float32)
        valid = perm >= 0
        rowpos = np.nonzero(valid)[0]
        # support-matmul column for node (p, b) is b*128+p, not p*NB+b
        cols = (rowpos % NB) * P + rowpos // NB
        xt[:, cols] = x[lo + perm[rowpos]].T

        in_maps.append(dict(
            xt=xt, wp=wp,
            idxg=idxg,
            w1c=w1c.T.astype(bf16).copy(),     # [128 slots, T]
            w1v=w1v.T.astype(bf16).copy(),
            iot=iot.astype(bf16),
        ))

    meta = dict(T=T, TLs=TLs, NB=NB, SP=SP, SHARD=SHARD, HALF=HALF,
                chunks=chunks, sched=sched, perms=perms)
    return in_maps, meta


# ----------------------------------------------------------------------------
# Device kernel
# ----------------------------------------------------------------------------

def _build(cfg, meta, sim=False):
    import concourse.bacc as bacc
    import concourse.tile as tile
    from concourse import bass, mybir
    from concourse.library_config import mlp

    IN_F, ITERS, C = cfg["IN_F"], cfg["ITERS"], cfg["CORES"]
    ALPHA = cfg["ALPHA"]
    T, NB, SP, HALF = meta["T"], meta["NB"], meta["SP"], meta["HALF"]
    chunks, sched = meta["chunks"], meta["sched"]
    KC = IN_F // P
    F = OUT_F
    n_ch = len(chunks)
    IDXW = sum(n * P // 16 for _, n, _ in chunks)

    nc = bacc.Bacc("TRN2", target_bir_lowering=False,
                   num_devices=1 if sim else C)

    xt_d = nc.dram_tensor("xt", [IN_F, SP], mybir.dt.float32,
                          kind="ExternalInput")
    wp_d = nc.dram_tensor("wp", [P, KC * F], mybir.dt.float32,
                          kind="ExternalInput")
    idxg_d = nc.dram_tensor("idxg", [P, IDXW], mybir.dt.int16,
                            kind="ExternalInput")
    w1c_d = nc.dram_tensor("w1c", [P, T], mybir.dt.bfloat16,
                           kind="ExternalInput")
    w1v_d = nc.dram_tensor("w1v", [P, T], mybir.dt.bfloat16,
                           kind="ExternalInput")
    iot_d = nc.dram_tensor("iot", [P, P], mybir.dt.bfloat16,
                           kind="ExternalInput")
    out_d = nc.dram_tensor("out", [P, NB * F], mybir.dt.float32,
                           kind="ExternalOutput")

    RG = [list(range(C))]

    with tile.TileContext(nc) as tc:
        with (
            tc.tile_pool(name="const", bufs=1) as cpool,
            tc.tile_pool(name="dramsh", bufs=2, space="DRAM") as shpool,
            tc.tile_pool(name="zp", bufs=3) as zpool,
            tc.tile_pool(name="w1p", bufs=3) as w1pool,
            tc.tile_pool(name="pp", bufs=4, space="PSUM") as ppool,
        ):
            nc.gpsimd.load_library(mlp)

            idxg_sb = cpool.tile([P, IDXW], mybir.dt.int16)
            nc.sync.dma_start(out=idxg_sb[:], in_=idxg_d[:])
            w1c_sb = cpool.tile([P, T], mybir.dt.bfloat16, name="w1c_sb")
            w1v_sb = cpool.tile([P, T], mybir.dt.bfloat16, name="w1v_sb")
            iot_sb = cpool.tile([P, P], mybir.dt.bfloat16, name="iot_sb")
            nc.sync.dma_start(out=w1c_sb[:], in_=w1c_d[:])
            nc.sync.dma_start(out=w1v_sb[:], in_=w1v_d[:])
            nc.sync.dma_start(out=iot_sb[:], in_=iot_d[:])

            s01_sb = cpool.tile([P, NB * F], mybir.dt.float32)   # 0.1*support
            acc_sb = cpool.tile([P, NB * F], mybir.dt.float32)   # lo partials
            d_sb = cpool.tile([P, NB * F], mybir.dt.float32)
            hn_bf = cpool.tile([P, NB * FW], mybir.dt.bfloat16)
            nc.vector.memset(hn_bf[:], 0.0)

            s01v = s01_sb[:].rearrange("p (c f) -> p c f", f=F)
            accv = acc_sb[:].rearrange("p (c f) -> p c f", f=F)
            dv = d_sb[:].rearrange("p (c f) -> p c f", f=F)
            hnv = hn_bf[:].rearrange("p (c f) -> p c f", f=FW)[:, :, :F]

            # --- support = x @ W: s01 = 0.1*support, hn = bf16(support)
            with (
                tc.tile_pool(name="xtp", bufs=2) as xtp,
                tc.tile_pool(name="spp", bufs=2, space="PSUM") as spp,
                tc.tile_pool(name="scp", bufs=1) as scp,
            ):
                wp_sb = scp.tile([P, KC * F], mybir.dt.float32, name="wp_sb")
                nc.sync.dma_start(out=wp_sb[:], in_=wp_d[:])
                MG = 4
                for mq in range(0, NB, MG):
                    mw = min(MG, NB - mq)
                    xm = xtp.tile([P, KC * MG * P], mybir.dt.float32,
                                  name="xm", tag="xm")
                    for k in range(KC):
                        nc.sync.dma_start(
                            out=xm[:, k * MG * P:k * MG * P + mw * P],
                            in_=xt_d[k * P:(k + 1) * P, mq * P:(mq + mw) * P])
                    for mi in range(mw):
                        m = mq + mi
                        ps = spp.tile([P, F], mybir.dt.float32, name="ps_sup",
                                      tag="ps_sup")
                        for k in range(KC):
                            nc.tensor.matmul(
                                out=ps[:],
                                lhsT=xm[:, (k * MG + mi) * P:(k * MG + mi + 1) * P],
                                rhs=wp_sb[:, k * F:(k + 1) * F],
                                start=(k == 0), stop=(k == KC - 1))
                        if m % 2 == 0:
                            nc.vector.tensor_copy(out=hnv[:, m, :], in_=ps[:])
                        else:
                            nc.scalar.copy(out=hnv[:, m, :], in_=ps[:])
                        nc.vector.tensor_scalar_mul(s01v[:, m, :], ps[:], ALPHA)

            # --- iterations
            agin = shpool.tile([SP, FW], mybir.dt.bfloat16, name="agin",
                               tag="agin", bufs=2)
            nc.sync.dma_start(out=agin[:], in_=hn_bf[:])
            for it in range(ITERS):
                if it + 1 < ITERS:
                    # next iteration's AG input, streamed by hi epilogues
                    agin_next = shpool.tile([SP, FW], mybir.dt.bfloat16,
                                            name="agin", tag="agin", bufs=2)
                    agin_nv = agin_next[:].rearrange(
                        "(p b) f -> p b f", b=NB)
                else:
                    agin_next = None
                if sim:
                    hfull = shpool.tile([C * SP, FW], mybir.dt.bfloat16,
                                        name="hfull", tag="hfull", bufs=2)
                    for cc in range(C):
                        nc.sync.dma_start(
                            out=hfull[cc * SP:(cc + 1) * SP, :], in_=agin[:])
                else:
                    hfull = shpool.tile([C * SP, FW], mybir.dt.bfloat16,
                                        name="hfull", tag="hfull", bufs=2,
                                        addr_space="Shared")
                    nc.gpsimd.collective_compute(
                        "AllGather", mybir.AluOpType.bypass,
                        replica_groups=RG,
                        ins=[agin.opt()], outs=[hfull.opt()])

                # stream chunks: gather z, load w1, matmul per tile
                tpos = 0
                iw = 0
                for ci, (t0, ntl, half) in enumerate(chunks):
                    tab = hfull[0:HALF, :] if half == 0 else \
                        hfull[HALF:2 * HALF, :]
                    z = zpool.tile([P, CH_TILES, FW], mybir.dt.bfloat16,
                                   name="z", tag="z")
                    nc.gpsimd.dma_gather(
                        out_ap=z[:, :ntl, :], in_ap=tab,
                        idxs_ap=idxg_sb[:, iw:iw + ntl * P // 16],
                        num_idxs=ntl * P, num_idxs_reg=ntl * P,
                        elem_size=FW, single_packet=False)
                    iw += ntl * P // 16
                    # build one-hot lhsT on DVE: w1[s, t, c] =
                    #   (col[s,t]==c) * val[s,t]
                    w1t = w1pool.tile([P, CH_TILES, P], mybir.dt.bfloat16,
                                      name="w1t", tag="w1t")
                    cvb = w1c_sb[:, t0:t0 + ntl].rearrange(
                        "p (t o) -> p t o", o=1).to_broadcast([P, ntl, P])
                    vvb = w1v_sb[:, t0:t0 + ntl].rearrange(
                        "p (t o) -> p t o", o=1).to_broadcast([P, ntl, P])
                    iob = iot_sb[:].rearrange(
                        "p (o f) -> p o f", o=1).to_broadcast([P, ntl, P])
                    nc.vector.tensor_tensor(
                        out=w1t[:, :ntl, :], in0=cvb, in1=iob,
                        op=mybir.AluOpType.is_equal)
                    nc.vector.tensor_tensor(
                        out=w1t[:, :ntl, :], in0=w1t[:, :ntl, :], in1=vvb,
                        op=mybir.AluOpType.mult)
                    for ti in range(ntl):
                        t = t0 + ti
                        blk, st, sp_ = sched[t]
                        if st:
                            ps_cur = ppool.tile([P, F], mybir.dt.float32,
                                                name="ps", tag="ps")
                        ps = ps_cur
                        nc.tensor.matmul(
                            out=ps[:],
                            lhsT=w1t[:, ti, :],
                            rhs=z[:, ti, :F],
                            start=st, stop=sp_)
                        if sp_:
                            if t < meta["TLs"]:
                                # lo epilogue: acc = psum + s01
                                nc.vector.tensor_tensor(
                                    out=accv[:, blk, :], in0=ps[:],
                                    in1=s01v[:, blk, :],
                                    op=mybir.AluOpType.add)
                            else:
                                # hi epilogue: d = psum + acc; relu
                                nc.vector.tensor_tensor(
                                    out=dv[:, blk, :], in0=ps[:],
                                    in1=accv[:, blk, :],
                                    op=mybir.AluOpType.add)
                                if it < ITERS - 1:
                                    nc.scalar.activation(
                                        out=hnv[:, blk, :], in_=dv[:, blk, :],
                                        func=mybir.ActivationFunctionType.Relu)
                                    nc.sync.dma_start(
                                        out=agin_nv[:, blk, :],
                                        in_=hn_bf[:, blk * FW:(blk + 1) * FW])
                                else:
                                    nc.scalar.activation(
                                        out=dv[:, blk, :], in_=dv[:, blk, :],
                                        func=mybir.ActivationFunctionType.Relu)
                if it == ITERS - 1:
                    nc.sync.dma_start(out=out_d[:], in_=d_sb[:])
                agin = agin_next

    nc.compile()
    return nc


# ----------------------------------------------------------------------------
# Entry point
# ----------------------------------------------------------------------------

_CACHE = {}


def _run(inputs, cfg, profile=False, tmpdir=None):
    from concourse.bass_utils import run_bass_kernel_spmd

    in_maps, meta = _prep(inputs, cfg)
    key = (cfg["N"], cfg["E"], meta["T"], meta["TLs"])
    if key not in _CACHE:
        _CACHE[key] = _build(cfg, meta)
    nc = _CACHE[key]
    res = run_bass_kernel_spmd(nc, in_maps, core_ids=list(range(cfg["CORES"])),
                               trace=profile, tmpdir=tmpdir)
    outs = np.zeros((cfg["N"], OUT_F), np.float32)
    NB, SHARD, SP = meta["NB"], meta["SHARD"], meta["SP"]
    for c in range(cfg["CORES"]):
        arr = np.asarray(res.results[c]["out"])          # [128, NB*F]
        arr = arr.reshape(P, NB, OUT_F)
        perm = meta["perms"][c]
        valid = perm >= 0
        rows = np.nonzero(valid)[0]                      # table-row positions
        pp_, bb = rows // NB, rows % NB
        outs[c * SHARD + perm[valid]] = arr[pp_, bb]
    return (outs, res) if profile else outs


def kernel(**inputs) -> np.ndarray:
    return _run(inputs, _DEF)
